# revision 60
# baseline (speedup 1.0000x reference)
"""Trainium2 Bass kernel for nn_DavidBeansV2 (sparse wormhole attention).

Math (per batch item b, derived from the reference):
  xp = x[b, 1:, :]                                  # [P, D]
  q  = l2norm(xp @ Wq + bq); k = l2norm(xp @ Wk + bk)
  S  = q @ k.T + pos_bias    (diag forced very negative)
  topk16 per row of S/TEMP -> softmax weights w (zero elsewhere)
  v  = xp @ Wv + bv
  out[b] = (w / rowsum(w)) @ v                      # [P, D]
The multihead gather+combine with routes shared across heads is exactly a
row-sparse [P,P] x [P,D] matmul, so we compute it densely on the PE with a
masked-softmax weight matrix.

Sharding: data-parallel over batch B=8 across the 8 NeuronCores.

Precision: top-16 boundary gaps go down to ~1e-6, so the score path needs
fp32-grade accuracy, but fp32 matmuls run at 1/4 PE rate. Instead every
accuracy-critical matmul runs as ONE fp16 main pass (full rate) plus TWO
fp8e5 DoubleRow correction passes (0.25 pass each at the 0.5 cyc/row
double-pumped rate) accumulated into the same PSUM group:
    a @ b ~= fp16(a) @ fp16(b) + e5m2(da*s) @ e5m2(b/s)
                               + e5m2(a/t) @ e5m2(db*t)
with da = a - fp16(a) (host-side for x/W, on-chip for q/k) and per-tensor
power-of-two scales chosen so products land at scale 1 (so they can share
the PSUM group) while every fp8 operand stays in e5m2 range. Residual
error ~2e-5 relative per matmul (score noise ~1e-6, end-to-end rel err
~7e-3 from a handful of top-16 boundary flips) for 1.5x the single-pass
PE cost, vs 3x for classic compensated fp16 pairs. fp16 mains also halve
the input DMA bytes vs fp32 and need no on-chip rounding converts.

Normalization is factored out of q/k and applied around the scores: the
top-16 is selected on s'' = (qraw.kraw)*rk[col] + pb*|q[row]| (a
row-positive rescale of the true scores, so the same selection), and the
softmax exp recovers the true weights via its per-partition scale rq/T
and bias -m''*rq/T. pb*|q| is built off the critical chain.

The top-16 softmax mask uses exp cancellation: w = exp(..s''..) -
exp(..sz2..), where sz2 is s'' with the top-16 replaced by -50 via two
max8+match_replace rounds. Off-top entries produce identical fp16 values
and cancel exactly; on-top entries of the second exp underflow to ~0.
The [P,P] weight transpose for the combine runs on the DMA xbar, off the
PE. V projection and the combine run in plain fp16 (error smooth).
"""

import numpy as np
import ml_dtypes

import concourse.mybir as mybir
import concourse.tile as tile
from concourse import bass_isa
from concourse import bacc
from concourse.bass_utils import run_bass_kernel_spmd
from concourse.masks import make_identity

F32 = mybir.dt.float32
F16 = mybir.dt.float16
F8 = mybir.dt.float8e5
AF = mybir.ActivationFunctionType
OP = mybir.AluOpType
PM = mybir.MatmulPerfMode

B, P, D = 8, 1024, 768
TEMP = 0.1
KC = D // 128     # 6 contraction chunks
PB = P // 128     # 8 row blocks
MINVAL = -50.0    # match_replace fill; below any real s'', above diag fill
DIAGVAL = -10000.0

# fp8 correction scales: products must land at scale 1 to share the PSUM
# group, and every scaled operand must sit in e5m2's representable range.
# fp16 main residuals are ~2^-11 relative.
SX = 32.0         # dx8 = e5(dx*SX),  w8  = e5(W/SX)
SW = 2048.0       # dw8 = e5(dW*SW),  x8  = e5(x/SW)
SQ = 32.0         # dq8 = e5(dq*SQ),  k8  = e5(k/SQ)
SK = 2048.0       # dk8 = e5(dk*SK),  q8  = e5(q/SK)


def build_program(with_bias: bool):
    nc = bacc.Bacc(
        "TRN2",
        target_bir_lowering=False,
        debug=False,
        enable_asserts=False,
        num_devices=B,
    )
    xT = nc.dram_tensor("xT", [D, P], F16, kind="ExternalInput").ap()
    x8_d = nc.dram_tensor("x8_d", [D, P], F8, kind="ExternalInput").ap()
    dx8_d = nc.dram_tensor("dx8_d", [D, P], F8, kind="ExternalInput").ap()
    Wq_d = nc.dram_tensor("Wq_d", [D, D], F16, kind="ExternalInput").ap()
    Wk_d = nc.dram_tensor("Wk_d", [D, D], F16, kind="ExternalInput").ap()
    Wv_d = nc.dram_tensor("Wv_d", [D, D], F16, kind="ExternalInput").ap()
    w8q_d = nc.dram_tensor("w8q_d", [D, D], F8, kind="ExternalInput").ap()
    dw8q_d = nc.dram_tensor("dw8q_d", [D, D], F8, kind="ExternalInput").ap()
    w8k_d = nc.dram_tensor("w8k_d", [D, D], F8, kind="ExternalInput").ap()
    dw8k_d = nc.dram_tensor("dw8k_d", [D, D], F8, kind="ExternalInput").ap()
    pb = nc.dram_tensor("pb", [P, P], F32, kind="ExternalInput").ap()
    if with_bias:
        bqkv = nc.dram_tensor("bqkv", [1, 3, D], F32, kind="ExternalInput").ap()
    out = nc.dram_tensor("out", [P, D], F32, kind="ExternalOutput").ap()

    def ch(ap):  # [(o p), f] -> partition-chunked view
        return ap.rearrange("(o p) f -> p o f", p=128)

    halves = ((0, slice(0, 512)), (1, slice(512, P)))
    dhalves = ((0, slice(0, 512)), (1, slice(512, D)))

    with tile.TileContext(nc) as tc:
        consts = tc.alloc_tile_pool(name="consts", bufs=1)
        persist = tc.alloc_tile_pool(name="persist", bufs=1)
        inp_pool = tc.alloc_tile_pool(name="inp", bufs=1)
        wpool = tc.alloc_tile_pool(name="wpool", bufs=1)
        work2 = tc.alloc_tile_pool(name="work2", bufs=1)
        tmp_pool = tc.alloc_tile_pool(name="tmp", bufs=1)
        psum2 = tc.alloc_tile_pool(name="psum2", bufs=1, space="PSUM")

        ident = consts.tile([128, 128], F16, tag="ident")
        make_identity(nc, ident)
        # warm-up matmuls: keep the PE busy through the initial input-DMA
        # wait so the HAM clock gate is at full rate when real work starts
        warm_ps = psum2.tile([1, 64], F32, tag="warm_ps")
        for _ in range(150):
            nc.tensor.matmul(warm_ps, ident[:, 0:1], ident[:, 0:64],
                             start=True, stop=True)
        ones_row = consts.tile([1, 128], F32, tag="ones_row")
        nc.vector.memset(ones_row, 1.0)

        # ---- input loads: one DMA per tensor, in consumption order ----
        # (the model's DMA transfers serialize, so order = priority)
        x1 = inp_pool.tile([128, KC, P], F16, tag="x1", name="x1")
        x8 = inp_pool.tile([128, KC, P], F8, tag="x8", name="x8")
        dx8 = inp_pool.tile([128, KC, P], F8, tag="dx8", name="dx8")
        wv1 = wpool.tile([128, KC, D], F16, tag="wv1", name="wv1")

        def alloc_w(nm):
            w1 = wpool.tile([128, KC, D], F16, tag=f"w1{nm}", name=f"w1{nm}")
            w8 = wpool.tile([128, KC, D], F8, tag=f"w8{nm}", name=f"w8{nm}")
            dw8 = wpool.tile([128, KC, D], F8, tag=f"dw8{nm}", name=f"dw8{nm}")
            return w1, w8, dw8

        w1q, w8q, dw8q = alloc_w("q")
        w1k, w8k, dw8k = alloc_w("k")
        # chunked, interleaved head loads so the V projection's first
        # accumulation steps start as soon as their x/wv chunks land
        nc.sync.dma_start(x1[:, 0, :], ch(xT)[:, 0, :])
        nc.sync.dma_start(wv1[:, 0:3, :], ch(Wv_d)[:, 0:3, :])
        nc.sync.dma_start(x1[:, 1, :], ch(xT)[:, 1, :])
        nc.sync.dma_start(x1[:, 2, :], ch(xT)[:, 2, :])
        nc.sync.dma_start(wv1[:, 3:6, :], ch(Wv_d)[:, 3:6, :])
        for dc in range(3, KC):
            nc.sync.dma_start(x1[:, dc, :], ch(xT)[:, dc, :])
        nc.sync.dma_start(w1k, ch(Wk_d))
        nc.sync.dma_start(x8, ch(x8_d))
        nc.sync.dma_start(dx8, ch(dx8_d))
        nc.sync.dma_start(w8k, ch(w8k_d))
        nc.sync.dma_start(dw8k, ch(dw8k_d))
        nc.sync.dma_start(w1q, ch(Wq_d))
        nc.sync.dma_start(w8q, ch(w8q_d))
        nc.sync.dma_start(dw8q, ch(dw8q_d))
        if with_bias:
            bias32 = consts.tile([1, 3, D], F32, tag="bias32")
            nc.sync.dma_start(bias32, bqkv)
            bias16 = consts.tile([1, 3, D], F16, tag="bias16")
            nc.scalar.activation(bias16, bias32, AF.Identity)
            ones16 = consts.tile([1, 128], F16, tag="ones16")
            nc.vector.memset(ones16, 1.0)

        # ---- v projection (fp16; x1 stationary, wv1 moving) ----
        v_sb = persist.tile([128, PB, D], F16, tag="v_sb")
        for pblk in range(PB):
            mm_ps = psum2.tile([128, P], F32, tag="mm_out", name="mm_ps",
                               bufs=3)
            for dc in range(KC):
                for sl, s in dhalves:
                    nc.tensor.matmul(
                        mm_ps[:, s],
                        x1[:, dc, pblk * 128:(pblk + 1) * 128],
                        wv1[:, dc, s],
                        start=(dc == 0),
                        stop=(dc == KC - 1) and not with_bias,
                    )
            if with_bias:
                for sl, s in dhalves:
                    nc.tensor.matmul(
                        mm_ps[:, s],
                        ones16,
                        bias16[:, 2, s],
                        start=False,
                        stop=True,
                    )
            nc.scalar.activation(v_sb[:, pblk, :], mm_ps[:, :D], AF.Identity)

        # ---- q/k raw projections: fp16 main + fp8 corrections + norms ----
        p1 = {}
        p8 = {}
        dp8 = {}

        def emit_proj(ti, nm, w1, w8, dw8, interleave_list=None,
                      self_defer=False):
            deferred = []
            t1 = persist.tile([128, KC, P], F16, tag=f"{nm}1", name=f"{nm}1")
            t8 = persist.tile([128, KC, P], F8, tag=f"{nm}8", name=f"{nm}8")
            dt8 = persist.tile([128, KC, P], F8, tag=f"d{nm}8", name=f"d{nm}8")
            p1[nm], p8[nm], dp8[nm] = t1, t8, dt8
            s_in = SK if nm == "q" else SQ    # low-res copy scale (1/s_in)
            s_d = SQ if nm == "q" else SK     # residual scale
            sq_acc = work2.tile([128, P], F32, tag="sq_acc")
            for dblk in range(KC):
                dbs = slice(dblk * 128, (dblk + 1) * 128)
                mm_ps = psum2.tile([128, P], F32, tag="mm_out", name="mm_ps",
                                   bufs=3)
                for dc in range(KC):
                    for sl, s in halves:
                        nc.tensor.matmul(
                            mm_ps[:, s],
                            w1[:, dc, dbs],
                            x1[:, dc, s],
                            start=(dc == 0),
                            stop=False,
                        )
                for i in range(KC // 2):
                    c2 = slice(2 * i, 2 * i + 2)
                    for sl, s in halves:
                        nc.tensor.matmul(
                            mm_ps[:, s],
                            w8[:, c2, dbs],
                            dx8[:, c2, s],
                            start=False, stop=False, perf_mode=PM.DoubleRow,
                        )
                for i in range(KC // 2):
                    c2 = slice(2 * i, 2 * i + 2)
                    last = (i == KC // 2 - 1) and not with_bias
                    for sl, s in halves:
                        nc.tensor.matmul(
                            mm_ps[:, s],
                            dw8[:, c2, dbs],
                            x8[:, c2, s],
                            start=False, stop=last,
                            perf_mode=PM.DoubleRow,
                        )
                if with_bias:
                    for sl, s in halves:
                        nc.tensor.matmul(
                            mm_ps[:, s],
                            bias16[:, ti, dbs],
                            ones16,
                            start=False,
                            stop=True,
                        )
                # split: fp16 high part, fp8 residual + fp8 low-res copy,
                # squares for the row norms (from the fp16 copy: its 2^-11
                # rounding is far inside the norm's error budget)
                nc.scalar.activation(t1[:, dblk, :], mm_ps, AF.Identity)
                d_sb = tmp_pool.tile([128, P], F16, tag=f"tmp_{nm}",
                                     name="d", bufs=KC + 1)
                nc.vector.tensor_sub(d_sb, mm_ps, t1[:, dblk, :])

                # the fp8 conversions are off-critical (needed only by the
                # score matmuls much later): defer them into windows where
                # ACT has slack, so they never delay the PSUM drain or the
                # next phase behind them in the ACT FIFO
                def _mk(g, d):
                    def _cv():
                        nc.scalar.activation(t8[:, g, :], t1[:, g, :],
                                             AF.Identity, scale=1.0 / s_in)
                        nc.scalar.activation(dt8[:, g, :], d, AF.Identity,
                                             scale=s_d)
                    return _cv
                deferred.append(_mk(dblk, d_sb))
                if interleave_list:
                    interleave_list.pop(0)()
                elif self_defer and len(deferred) >= 2:
                    deferred.pop(0)()
                if dblk == 0:
                    nc.gpsimd.tensor_mul(sq_acc, t1[:, dblk, :], t1[:, dblk, :])
                else:
                    sq_sb = tmp_pool.tile([128, P], F32, tag="tmp2", name="sq",
                                          bufs=2)
                    nc.gpsimd.tensor_mul(sq_sb, t1[:, dblk, :], t1[:, dblk, :])
                    nc.vector.tensor_add(sq_acc, sq_acc, sq_sb)
            # norm2 = sum over partitions of sq_acc (gpsimd tree reduce)
            if nm == "q":
                allr = persist.tile([128, P], F32, tag="allr_q", name="allr_q")
            else:
                allr = work2.tile([128, P], F32, tag="allr_k", name="allr_k")
            nc.gpsimd.partition_all_reduce(allr, sq_acc, channels=128,
                                           reduce_op=bass_isa.ReduceOp.add)
            return allr, deferred

        def newton_rsqrt(r, n2, tmp, steps=2):
            # r <- r * (1.5 - 0.5 * n2 * r^2), refining an Rsqrt spline seed
            for _ in range(steps):
                nc.vector.tensor_mul(tmp, r, r)
                nc.vector.tensor_mul(tmp, tmp, n2)
                nc.vector.tensor_scalar(tmp, tmp, -0.5, 1.5, op0=OP.mult,
                                        op1=OP.add)
                nc.vector.tensor_mul(r, r, tmp)

        # ---- K first: its row-wise norm chain (Rsqrt spline + 2 Newton
        # steps) and the rk broadcast hide under Q's matmuls ----
        allr_k, defer_k = emit_proj(1, "k", w1k, w8k, dw8k)
        norm2_k = allr_k[0:1, :]
        sk_row = work2.tile([1, P], F32, tag="sk_row")
        nc.scalar.activation(sk_row, norm2_k, AF.Sqrt)
        rinv_k = work2.tile([1, P], F32, tag="rinv_k")
        nc.vector.reciprocal(rinv_k, sk_row)
        rr_row = work2.tile([1, P], F32, tag="rr_row")
        newton_rsqrt(rinv_k, norm2_k, rr_row)
        rk_bcast = persist.tile([128, P], F32, tag="rk_bcast")
        nc.gpsimd.partition_broadcast(rk_bcast, rinv_k, channels=128)

        # ---- Q second; only its partition-reduce runs here. The rq/|q|
        # relayout + Newton run on tiny [128, PB] tiles, emitted inside
        # phase 2 after block 0's score matmuls (off the PE stream). ----
        allr_q, defer_q = emit_proj(0, "q", w1q, w8q, dw8q,
                                    interleave_list=defer_k,
                                    self_defer=True)
        for cv in defer_k:   # any K converts not soaked up by Q's loop
            cv()
        for cv in defer_q:   # trailing Q converts (one-deferred in-loop)
            cv()

        tmp_pool.release()
        work2.release()
        wpool.release()
        inp_pool.release()
        psum2.release()

        # ---- per row-block: scores, top-16 softmax, combine ----
        q1, q8, dq8 = p1["q"], p8["q"], dp8["q"]
        k1, k8, dk8 = p1["k"], p8["k"], dp8["k"]
        work3 = tc.alloc_tile_pool(name="work3", bufs=2)
        psum3 = tc.alloc_tile_pool(name="psum3", bufs=1, space="PSUM")

        # prefetch all pos_bias blocks up front so the per-block DMA queues
        # are free for the w transposes and output stores
        pb_tiles = []
        for pblk in range(PB):
            pb_sb = work3.tile([128, P], F32, tag="pb_sb", name="pb_sb",
                               bufs=PB)
            nc.sync.dma_start(pb_sb, pb[pblk * 128:(pblk + 1) * 128, :])
            pb_tiles.append(pb_sb)

        # deferred q-side relayout: norm^2 -> per-partition columns via tiny
        # PE matmuls, then Rsqrt + Newton on [128, PB] tiles (sub-us ops)
        rq_cols = persist.tile([128, PB], F32, tag="rq_cols")
        nq_cols = persist.tile([128, PB], F32, tag="nq_cols")
        rqT = persist.tile([128, PB], F32, tag="rqT")
        rqTn = persist.tile([128, PB], F32, tag="rqTn")

        def emit_q_relayout():
            rq_ps = psum3.tile([128, PB], F32, tag="rq_ps", name="rq_ps",
                               bufs=1)
            for j in range(PB):
                nc.tensor.matmul(
                    rq_ps[:, j:j + 1],
                    allr_q[0:1, j * 128:(j + 1) * 128],
                    ones_row[:, 0:1],
                    start=True,
                    stop=True,
                )
            n2c = work3.tile([128, PB], F32, tag="n2c")
            nc.scalar.activation(n2c, rq_ps, AF.Identity)
            sqc = work3.tile([128, PB], F32, tag="sqc")
            nc.scalar.activation(sqc, n2c, AF.Sqrt)
            nc.vector.reciprocal(rq_cols, sqc)
            rrc = work3.tile([128, PB], F32, tag="rrc")
            newton_rsqrt(rq_cols, n2c, rrc)
            nc.vector.tensor_mul(nq_cols, n2c, rq_cols)
            nc.vector.tensor_scalar_mul(rqT, rq_cols, 1.0 / TEMP)
            nc.vector.tensor_scalar_mul(rqTn, rq_cols, -1.0 / TEMP)

        def emit_scores_a(pblk):
            """Score matmuls + the PSUM->SBUF rk multiply and pb add."""
            pbs = slice(pblk * 128, (pblk + 1) * 128)
            s_ps = psum3.tile([128, P], F32, tag="s_ps", name="s_ps", bufs=2)
            pb_sb = pb_tiles[pblk]
            for sl, s in halves:
                for dc in range(KC):
                    nc.tensor.matmul(
                        s_ps[:, s],
                        q1[:, dc, pbs],
                        k1[:, dc, s],
                        start=(dc == 0),
                        stop=False,
                    )
                for i in range(KC // 2):
                    c2 = slice(2 * i, 2 * i + 2)
                    nc.tensor.matmul(
                        s_ps[:, s],
                        dq8[:, c2, pbs],
                        k8[:, c2, s],
                        start=False, stop=False, perf_mode=PM.DoubleRow,
                    )
                for i in range(KC // 2):
                    c2 = slice(2 * i, 2 * i + 2)
                    nc.tensor.matmul(
                        s_ps[:, s],
                        q8[:, c2, pbs],
                        dk8[:, c2, s],
                        start=False, stop=(i == KC // 2 - 1),
                        perf_mode=PM.DoubleRow,
                    )
            if pblk == 0:
                emit_q_relayout()
            # pb * |q| runs off the critical chain (pb lands early via DMA)
            pb_q = work3.tile([128, P], F32, tag="pb_q")
            nc.scalar.activation(pb_q, pb_sb, AF.Identity,
                                 scale=nq_cols[:, pblk:pblk + 1])
            s_sb = work3.tile([128, P], F32, tag="s_sb")
            nc.vector.tensor_mul(s_sb, s_ps, rk_bcast)
            nc.gpsimd.tensor_add(s_sb, s_sb, pb_q)
            return s_sb

        def emit_scores_b(pblk, s_sb):
            """Top-16 + masked softmax + DMA transpose; returns (wT, rden).

            Emitted one block behind stage A so the DVE FIFO never has a
            stage-B op (waiting on Pool/ACT) ahead of the next block's
            PSUM-draining multiply.
            """
            # top-16 per row: two rounds of max8 + match_replace
            m8a = work3.tile([128, 8], F32, tag="m8a")
            nc.vector.max(m8a, s_sb)
            ebias = work3.tile([128, 1], F32, tag="ebias")
            nc.vector.tensor_mul(ebias, m8a[:, 0:1], rqTn[:, pblk:pblk + 1])
            sz1 = work3.tile([128, P], F32, tag="sz1")
            nc.vector.match_replace(sz1, in_to_replace=m8a, in_values=s_sb,
                                    imm_value=MINVAL)
            m8b = work3.tile([128, 8], F32, tag="m8b")
            nc.vector.max(m8b, sz1)
            sz2 = work3.tile([128, P], F32, tag="sz2")
            nc.vector.match_replace(sz2, in_to_replace=m8b, in_values=sz1,
                                    imm_value=MINVAL)
            # exp cancellation: w = exp((s''-m'')*rq/T) - exp((sz2-m'')*rq/T).
            # Off-top entries produce identical fp16 values and cancel
            # exactly; on-top entries of the second exp underflow to ~0.
            w_all = work3.tile([128, P], F16, tag="w_all")
            d_all = work3.tile([128, 1], F32, tag="d_all")
            nc.scalar.activation(w_all, s_sb, AF.Exp, bias=ebias,
                                 scale=rqT[:, pblk:pblk + 1], accum_out=d_all)
            w_z = work3.tile([128, P], F16, tag="w_z")
            d_z = work3.tile([128, 1], F32, tag="d_z")
            nc.scalar.activation(w_z, sz2, AF.Exp, bias=ebias,
                                 scale=rqT[:, pblk:pblk + 1], accum_out=d_z)
            w_sb = work3.tile([128, P], F16, tag="w_sb")
            nc.gpsimd.tensor_sub(w_sb, w_all, w_z)
            den = work3.tile([128, 1], F32, tag="den")
            nc.gpsimd.tensor_sub(den, d_all, d_z)
            rden = work3.tile([128, 1], F32, tag="rden", bufs=6)
            nc.vector.reciprocal(rden, den)
            # transpose w on the DMA xbar (off the PE)
            wT_sb = work3.tile([128, PB, 128], F16, tag="wT_sb", bufs=6)
            nc.scalar.dma_start_transpose(wT_sb, w_sb)
            return wT_sb, rden

        def emit_tail(pblk, wT_sb, rden):
            """PE tail: combine with v, scale, store."""
            pbs = slice(pblk * 128, (pblk + 1) * 128)
            o_ps = psum3.tile([128, D], F32, tag="o_ps", name="o_ps", bufs=1)
            for qc in range(PB):
                for sl, s in dhalves:
                    nc.tensor.matmul(
                        o_ps[:, s],
                        wT_sb[:, qc, :],
                        v_sb[:, qc, s],
                        start=(qc == 0),
                        stop=(qc == PB - 1),
                    )
            out_sb = work3.tile([128, D], F32, tag="out_sb")
            nc.scalar.activation(out_sb, o_ps, AF.Identity, scale=rden)
            nc.sync.dma_start(out[pbs, :], out_sb)

        # software pipeline: stage B (top-k chain) runs one block behind
        # stage A (matmuls), and block p's combine is emitted five A-stages
        # later, so the whole vector chain has ample slack before the PE
        # needs its weights.
        TLAG = 5
        sA = [None] * PB
        done = [None] * PB
        for pblk in range(PB):
            sA[pblk] = emit_scores_a(pblk)
            if pblk >= 1:
                done[pblk - 1] = emit_scores_b(pblk - 1, sA[pblk - 1])
            if pblk >= TLAG:
                emit_tail(pblk - TLAG, *done[pblk - TLAG])
        done[PB - 1] = emit_scores_b(PB - 1, sA[PB - 1])
        for pblk in range(PB - TLAG, PB):
            emit_tail(pblk, *done[pblk])

        work3.release()
        psum3.release()
        persist.release()
        consts.release()

    nc.finalize()
    return nc


_PROG_CACHE = {}


def _e5(x):
    return np.ascontiguousarray(np.asarray(x, np.float32).astype(
        ml_dtypes.float8_e5m2))


def kernel(**inputs) -> np.ndarray:
    x = np.ascontiguousarray(np.asarray(inputs["x"], dtype=np.float32))
    Wq = np.asarray(inputs["Wq"], dtype=np.float32)
    Wk = np.asarray(inputs["Wk"], dtype=np.float32)
    Wv = np.asarray(inputs["Wv"], dtype=np.float32)
    bq = np.asarray(inputs["bq"], dtype=np.float32)
    bk = np.asarray(inputs["bk"], dtype=np.float32)
    bv = np.asarray(inputs["bv"], dtype=np.float32)
    pos_bias = np.asarray(inputs["pos_bias"], dtype=np.float32)

    with_bias = bool(np.any(bq) or np.any(bk) or np.any(bv))

    # Diagonal is excluded by the reference (set to -1e9 before top-k); any
    # value below every real score gives the identical top-16 and weights.
    pb_adj = np.ascontiguousarray(pos_bias.copy())
    np.fill_diagonal(pb_adj, DIAGVAL)

    if with_bias not in _PROG_CACHE:
        _PROG_CACHE[with_bias] = build_program(with_bias)
    nc = _PROG_CACHE[with_bias]

    def wpack(W):
        W1 = W.astype(np.float16)
        dW = W - W1.astype(np.float32)
        return (np.ascontiguousarray(W1), _e5(W * (1.0 / SX)), _e5(dW * SW))

    Wq16, w8q, dw8q = wpack(Wq)
    Wk16, w8k, dw8k = wpack(Wk)
    Wv16 = np.ascontiguousarray(Wv.astype(np.float16))

    in_maps = []
    for b in range(B):
        xTb = np.ascontiguousarray(x[b, 1:, :].T)
        x16 = xTb.astype(np.float16)
        dxb = xTb - x16.astype(np.float32)
        m = {
            "xT": np.ascontiguousarray(x16),
            "x8_d": _e5(xTb * (1.0 / SW)),
            "dx8_d": _e5(dxb * SX),
            "Wq_d": Wq16, "Wk_d": Wk16, "Wv_d": Wv16,
            "w8q_d": w8q, "dw8q_d": dw8q, "w8k_d": w8k, "dw8k_d": dw8k,
            "pb": pb_adj,
        }
        if with_bias:
            m["bqkv"] = np.ascontiguousarray(np.stack([bq, bk, bv])[None])
        in_maps.append(m)

    res = run_bass_kernel_spmd(nc, in_maps, core_ids=list(range(B)))
    return np.stack([res.results[b]["out"] for b in range(B)]).astype(np.float32)


# revision 61
# speedup vs baseline: 1.0021x; 1.0021x over previous
"""Trainium2 Bass kernel for nn_DavidBeansV2 (sparse wormhole attention).

Math (per batch item b, derived from the reference):
  xp = x[b, 1:, :]                                  # [P, D]
  q  = l2norm(xp @ Wq + bq); k = l2norm(xp @ Wk + bk)
  S  = q @ k.T + pos_bias    (diag forced very negative)
  topk16 per row of S/TEMP -> softmax weights w (zero elsewhere)
  v  = xp @ Wv + bv
  out[b] = (w / rowsum(w)) @ v                      # [P, D]
The multihead gather+combine with routes shared across heads is exactly a
row-sparse [P,P] x [P,D] matmul, so we compute it densely on the PE with a
masked-softmax weight matrix.

Sharding: data-parallel over batch B=8 across the 8 NeuronCores.

Precision: top-16 boundary gaps go down to ~1e-6, so the score path needs
fp32-grade accuracy, but fp32 matmuls run at 1/4 PE rate. Instead every
accuracy-critical matmul runs as ONE fp16 main pass (full rate) plus TWO
fp8e5 DoubleRow correction passes (0.25 pass each at the 0.5 cyc/row
double-pumped rate) accumulated into the same PSUM group:
    a @ b ~= fp16(a) @ fp16(b) + e5m2(da*s) @ e5m2(b/s)
                               + e5m2(a/t) @ e5m2(db*t)
with da = a - fp16(a) (host-side for x/W, on-chip for q/k) and per-tensor
power-of-two scales chosen so products land at scale 1 (so they can share
the PSUM group) while every fp8 operand stays in e5m2 range. Residual
error ~2e-5 relative per matmul (score noise ~1e-6, end-to-end rel err
~7e-3 from a handful of top-16 boundary flips) for 1.5x the single-pass
PE cost, vs 3x for classic compensated fp16 pairs. fp16 mains also halve
the input DMA bytes vs fp32 and need no on-chip rounding converts.

Normalization is factored out of q/k and applied around the scores: the
top-16 is selected on s'' = (qraw.kraw)*rk[col] + pb*|q[row]| (a
row-positive rescale of the true scores, so the same selection), and the
softmax exp recovers the true weights via its per-partition scale rq/T
and bias -m''*rq/T. pb*|q| is built off the critical chain.

The top-16 softmax mask uses exp cancellation: w = exp(..s''..) -
exp(..sz2..), where sz2 is s'' with the top-16 replaced by -50 via two
max8+match_replace rounds. Off-top entries produce identical fp16 values
and cancel exactly; on-top entries of the second exp underflow to ~0.
The [P,P] weight transpose for the combine runs on the DMA xbar, off the
PE. V projection and the combine run in plain fp16 (error smooth).
"""

import numpy as np
import ml_dtypes

import concourse.mybir as mybir
import concourse.tile as tile
from concourse import bass_isa
from concourse import bacc
from concourse.bass_utils import run_bass_kernel_spmd
from concourse.masks import make_identity

F32 = mybir.dt.float32
F16 = mybir.dt.float16
F8 = mybir.dt.float8e5
AF = mybir.ActivationFunctionType
OP = mybir.AluOpType
PM = mybir.MatmulPerfMode

B, P, D = 8, 1024, 768
TEMP = 0.1
KC = D // 128     # 6 contraction chunks
PB = P // 128     # 8 row blocks
MINVAL = -50.0    # match_replace fill; below any real s'', above diag fill
DIAGVAL = -10000.0

# fp8 correction scales: products must land at scale 1 to share the PSUM
# group, and every scaled operand must sit in e5m2's representable range.
# fp16 main residuals are ~2^-11 relative.
SX = 32.0         # dx8 = e5(dx*SX),  w8  = e5(W/SX)
SW = 2048.0       # dw8 = e5(dW*SW),  x8  = e5(x/SW)
SQ = 32.0         # dq8 = e5(dq*SQ),  k8  = e5(k/SQ)
SK = 2048.0       # dk8 = e5(dk*SK),  q8  = e5(q/SK)


def build_program(with_bias: bool):
    nc = bacc.Bacc(
        "TRN2",
        target_bir_lowering=False,
        debug=False,
        enable_asserts=False,
        num_devices=B,
    )
    xT = nc.dram_tensor("xT", [D, P], F16, kind="ExternalInput").ap()
    x8_d = nc.dram_tensor("x8_d", [D, P], F8, kind="ExternalInput").ap()
    dx8_d = nc.dram_tensor("dx8_d", [D, P], F8, kind="ExternalInput").ap()
    Wq_d = nc.dram_tensor("Wq_d", [D, D], F16, kind="ExternalInput").ap()
    Wk_d = nc.dram_tensor("Wk_d", [D, D], F16, kind="ExternalInput").ap()
    Wv_d = nc.dram_tensor("Wv_d", [D, D], F16, kind="ExternalInput").ap()
    w8q_d = nc.dram_tensor("w8q_d", [D, D], F8, kind="ExternalInput").ap()
    dw8q_d = nc.dram_tensor("dw8q_d", [D, D], F8, kind="ExternalInput").ap()
    w8k_d = nc.dram_tensor("w8k_d", [D, D], F8, kind="ExternalInput").ap()
    dw8k_d = nc.dram_tensor("dw8k_d", [D, D], F8, kind="ExternalInput").ap()
    pb = nc.dram_tensor("pb", [P, P], F32, kind="ExternalInput").ap()
    if with_bias:
        bqkv = nc.dram_tensor("bqkv", [1, 3, D], F32, kind="ExternalInput").ap()
    out = nc.dram_tensor("out", [P, D], F32, kind="ExternalOutput").ap()

    def ch(ap):  # [(o p), f] -> partition-chunked view
        return ap.rearrange("(o p) f -> p o f", p=128)

    halves = ((0, slice(0, 512)), (1, slice(512, P)))
    dhalves = ((0, slice(0, 512)), (1, slice(512, D)))

    with tile.TileContext(nc) as tc:
        consts = tc.alloc_tile_pool(name="consts", bufs=1)
        persist = tc.alloc_tile_pool(name="persist", bufs=1)
        inp_pool = tc.alloc_tile_pool(name="inp", bufs=1)
        wpool = tc.alloc_tile_pool(name="wpool", bufs=1)
        work2 = tc.alloc_tile_pool(name="work2", bufs=1)
        tmp_pool = tc.alloc_tile_pool(name="tmp", bufs=1)
        psum2 = tc.alloc_tile_pool(name="psum2", bufs=1, space="PSUM")

        ident = consts.tile([128, 128], F16, tag="ident")
        make_identity(nc, ident)
        # warm-up matmuls: keep the PE busy through the initial input-DMA
        # wait so the HAM clock gate is at full rate when real work starts
        warm_ps = psum2.tile([1, 64], F32, tag="warm_ps")
        for _ in range(120):
            nc.tensor.matmul(warm_ps, ident[:, 0:1], ident[:, 0:64],
                             start=True, stop=True)
        ones_row = consts.tile([1, 128], F32, tag="ones_row")
        nc.vector.memset(ones_row, 1.0)

        # ---- input loads: one DMA per tensor, in consumption order ----
        # (the model's DMA transfers serialize, so order = priority)
        x1 = inp_pool.tile([128, KC, P], F16, tag="x1", name="x1")
        x8 = inp_pool.tile([128, KC, P], F8, tag="x8", name="x8")
        dx8 = inp_pool.tile([128, KC, P], F8, tag="dx8", name="dx8")
        wv1 = wpool.tile([128, KC, D], F16, tag="wv1", name="wv1")

        def alloc_w(nm):
            w1 = wpool.tile([128, KC, D], F16, tag=f"w1{nm}", name=f"w1{nm}")
            w8 = wpool.tile([128, KC, D], F8, tag=f"w8{nm}", name=f"w8{nm}")
            dw8 = wpool.tile([128, KC, D], F8, tag=f"dw8{nm}", name=f"dw8{nm}")
            return w1, w8, dw8

        w1q, w8q, dw8q = alloc_w("q")
        w1k, w8k, dw8k = alloc_w("k")
        # chunked, interleaved head loads so the V projection's first
        # accumulation steps start as soon as their x/wv chunks land
        nc.sync.dma_start(x1[:, 0, :], ch(xT)[:, 0, :])
        nc.sync.dma_start(wv1[:, 0:3, :], ch(Wv_d)[:, 0:3, :])
        nc.sync.dma_start(x1[:, 1, :], ch(xT)[:, 1, :])
        nc.sync.dma_start(x1[:, 2, :], ch(xT)[:, 2, :])
        nc.sync.dma_start(wv1[:, 3:6, :], ch(Wv_d)[:, 3:6, :])
        for dc in range(3, KC):
            nc.sync.dma_start(x1[:, dc, :], ch(xT)[:, dc, :])
        nc.sync.dma_start(w1k, ch(Wk_d))
        nc.sync.dma_start(x8, ch(x8_d))
        nc.sync.dma_start(dx8, ch(dx8_d))
        nc.sync.dma_start(w8k, ch(w8k_d))
        nc.sync.dma_start(dw8k, ch(dw8k_d))
        nc.sync.dma_start(w1q, ch(Wq_d))
        nc.sync.dma_start(w8q, ch(w8q_d))
        nc.sync.dma_start(dw8q, ch(dw8q_d))
        if with_bias:
            bias32 = consts.tile([1, 3, D], F32, tag="bias32")
            nc.sync.dma_start(bias32, bqkv)
            bias16 = consts.tile([1, 3, D], F16, tag="bias16")
            nc.scalar.activation(bias16, bias32, AF.Identity)
            ones16 = consts.tile([1, 128], F16, tag="ones16")
            nc.vector.memset(ones16, 1.0)

        # ---- v projection (fp16; x1 stationary, wv1 moving) ----
        v_sb = persist.tile([128, PB, D], F16, tag="v_sb")
        for pblk in range(PB):
            mm_ps = psum2.tile([128, P], F32, tag="mm_out", name="mm_ps",
                               bufs=3)
            for dc in range(KC):
                for sl, s in dhalves:
                    nc.tensor.matmul(
                        mm_ps[:, s],
                        x1[:, dc, pblk * 128:(pblk + 1) * 128],
                        wv1[:, dc, s],
                        start=(dc == 0),
                        stop=(dc == KC - 1) and not with_bias,
                    )
            if with_bias:
                for sl, s in dhalves:
                    nc.tensor.matmul(
                        mm_ps[:, s],
                        ones16,
                        bias16[:, 2, s],
                        start=False,
                        stop=True,
                    )
            nc.scalar.activation(v_sb[:, pblk, :], mm_ps[:, :D], AF.Identity)

        # ---- q/k raw projections: fp16 main + fp8 corrections + norms ----
        p1 = {}
        p8 = {}
        dp8 = {}

        def emit_proj(ti, nm, w1, w8, dw8, interleave_list=None,
                      self_defer=False):
            deferred = []
            t1 = persist.tile([128, KC, P], F16, tag=f"{nm}1", name=f"{nm}1")
            t8 = persist.tile([128, KC, P], F8, tag=f"{nm}8", name=f"{nm}8")
            dt8 = persist.tile([128, KC, P], F8, tag=f"d{nm}8", name=f"d{nm}8")
            p1[nm], p8[nm], dp8[nm] = t1, t8, dt8
            s_in = SK if nm == "q" else SQ    # low-res copy scale (1/s_in)
            s_d = SQ if nm == "q" else SK     # residual scale
            sq_acc = work2.tile([128, P], F32, tag="sq_acc")
            for dblk in range(KC):
                dbs = slice(dblk * 128, (dblk + 1) * 128)
                mm_ps = psum2.tile([128, P], F32, tag="mm_out", name="mm_ps",
                                   bufs=3)
                for dc in range(KC):
                    for sl, s in halves:
                        nc.tensor.matmul(
                            mm_ps[:, s],
                            w1[:, dc, dbs],
                            x1[:, dc, s],
                            start=(dc == 0),
                            stop=False,
                        )
                for i in range(KC // 2):
                    c2 = slice(2 * i, 2 * i + 2)
                    for sl, s in halves:
                        nc.tensor.matmul(
                            mm_ps[:, s],
                            w8[:, c2, dbs],
                            dx8[:, c2, s],
                            start=False, stop=False, perf_mode=PM.DoubleRow,
                        )
                for i in range(KC // 2):
                    c2 = slice(2 * i, 2 * i + 2)
                    last = (i == KC // 2 - 1) and not with_bias
                    for sl, s in halves:
                        nc.tensor.matmul(
                            mm_ps[:, s],
                            dw8[:, c2, dbs],
                            x8[:, c2, s],
                            start=False, stop=last,
                            perf_mode=PM.DoubleRow,
                        )
                if with_bias:
                    for sl, s in halves:
                        nc.tensor.matmul(
                            mm_ps[:, s],
                            bias16[:, ti, dbs],
                            ones16,
                            start=False,
                            stop=True,
                        )
                # split: fp16 high part, fp8 residual + fp8 low-res copy,
                # squares for the row norms (from the fp16 copy: its 2^-11
                # rounding is far inside the norm's error budget)
                nc.scalar.activation(t1[:, dblk, :], mm_ps, AF.Identity)
                d_sb = tmp_pool.tile([128, P], F16, tag=f"tmp_{nm}",
                                     name="d", bufs=KC + 1)
                nc.vector.tensor_sub(d_sb, mm_ps, t1[:, dblk, :])

                # the fp8 conversions are off-critical (needed only by the
                # score matmuls much later): defer them into windows where
                # ACT has slack, so they never delay the PSUM drain or the
                # next phase behind them in the ACT FIFO
                def _mk(g, d):
                    def _cv():
                        nc.scalar.activation(t8[:, g, :], t1[:, g, :],
                                             AF.Identity, scale=1.0 / s_in)
                        nc.scalar.activation(dt8[:, g, :], d, AF.Identity,
                                             scale=s_d)
                    return _cv
                deferred.append(_mk(dblk, d_sb))
                if interleave_list:
                    interleave_list.pop(0)()
                elif self_defer and len(deferred) >= 2:
                    deferred.pop(0)()
                if dblk == 0:
                    nc.gpsimd.tensor_mul(sq_acc, t1[:, dblk, :], t1[:, dblk, :])
                else:
                    sq_sb = tmp_pool.tile([128, P], F32, tag="tmp2", name="sq",
                                          bufs=2)
                    nc.gpsimd.tensor_mul(sq_sb, t1[:, dblk, :], t1[:, dblk, :])
                    nc.vector.tensor_add(sq_acc, sq_acc, sq_sb)
            # norm2 = sum over partitions of sq_acc (gpsimd tree reduce)
            if nm == "q":
                allr = persist.tile([128, P], F32, tag="allr_q", name="allr_q")
            else:
                allr = work2.tile([128, P], F32, tag="allr_k", name="allr_k")
            nc.gpsimd.partition_all_reduce(allr, sq_acc, channels=128,
                                           reduce_op=bass_isa.ReduceOp.add)
            return allr, deferred

        def newton_rsqrt(r, n2, tmp, steps=2):
            # r <- r * (1.5 - 0.5 * n2 * r^2), refining an Rsqrt spline seed
            for _ in range(steps):
                nc.vector.tensor_mul(tmp, r, r)
                nc.vector.tensor_mul(tmp, tmp, n2)
                nc.vector.tensor_scalar(tmp, tmp, -0.5, 1.5, op0=OP.mult,
                                        op1=OP.add)
                nc.vector.tensor_mul(r, r, tmp)

        # ---- K first: its row-wise norm chain (Rsqrt spline + 2 Newton
        # steps) and the rk broadcast hide under Q's matmuls ----
        allr_k, defer_k = emit_proj(1, "k", w1k, w8k, dw8k)
        norm2_k = allr_k[0:1, :]
        sk_row = work2.tile([1, P], F32, tag="sk_row")
        nc.scalar.activation(sk_row, norm2_k, AF.Sqrt)
        rinv_k = work2.tile([1, P], F32, tag="rinv_k")
        nc.vector.reciprocal(rinv_k, sk_row)
        rr_row = work2.tile([1, P], F32, tag="rr_row")
        newton_rsqrt(rinv_k, norm2_k, rr_row)
        rk_bcast = persist.tile([128, P], F32, tag="rk_bcast")
        nc.gpsimd.partition_broadcast(rk_bcast, rinv_k, channels=128)

        # ---- Q second; only its partition-reduce runs here. The rq/|q|
        # relayout + Newton run on tiny [128, PB] tiles, emitted inside
        # phase 2 after block 0's score matmuls (off the PE stream). ----
        allr_q, defer_q = emit_proj(0, "q", w1q, w8q, dw8q,
                                    interleave_list=defer_k,
                                    self_defer=True)
        for cv in defer_k:   # any K converts not soaked up by Q's loop
            cv()
        for cv in defer_q:   # trailing Q converts (one-deferred in-loop)
            cv()

        tmp_pool.release()
        work2.release()
        wpool.release()
        inp_pool.release()
        psum2.release()

        # ---- per row-block: scores, top-16 softmax, combine ----
        q1, q8, dq8 = p1["q"], p8["q"], dp8["q"]
        k1, k8, dk8 = p1["k"], p8["k"], dp8["k"]
        work3 = tc.alloc_tile_pool(name="work3", bufs=2)
        psum3 = tc.alloc_tile_pool(name="psum3", bufs=1, space="PSUM")

        # prefetch all pos_bias blocks up front so the per-block DMA queues
        # are free for the w transposes and output stores
        pb_tiles = []
        for pblk in range(PB):
            pb_sb = work3.tile([128, P], F32, tag="pb_sb", name="pb_sb",
                               bufs=PB)
            nc.sync.dma_start(pb_sb, pb[pblk * 128:(pblk + 1) * 128, :])
            pb_tiles.append(pb_sb)

        # deferred q-side relayout: norm^2 -> per-partition columns via tiny
        # PE matmuls, then Rsqrt + Newton on [128, PB] tiles (sub-us ops)
        rq_cols = persist.tile([128, PB], F32, tag="rq_cols")
        nq_cols = persist.tile([128, PB], F32, tag="nq_cols")
        rqT = persist.tile([128, PB], F32, tag="rqT")
        rqTn = persist.tile([128, PB], F32, tag="rqTn")

        def emit_q_relayout():
            rq_ps = psum3.tile([128, PB], F32, tag="rq_ps", name="rq_ps",
                               bufs=1)
            for j in range(PB):
                nc.tensor.matmul(
                    rq_ps[:, j:j + 1],
                    allr_q[0:1, j * 128:(j + 1) * 128],
                    ones_row[:, 0:1],
                    start=True,
                    stop=True,
                )
            n2c = work3.tile([128, PB], F32, tag="n2c")
            nc.scalar.activation(n2c, rq_ps, AF.Identity)
            sqc = work3.tile([128, PB], F32, tag="sqc")
            nc.scalar.activation(sqc, n2c, AF.Sqrt)
            nc.vector.reciprocal(rq_cols, sqc)
            rrc = work3.tile([128, PB], F32, tag="rrc")
            newton_rsqrt(rq_cols, n2c, rrc)
            nc.vector.tensor_mul(nq_cols, n2c, rq_cols)
            nc.vector.tensor_scalar_mul(rqT, rq_cols, 1.0 / TEMP)
            nc.vector.tensor_scalar_mul(rqTn, rq_cols, -1.0 / TEMP)

        def emit_scores_a(pblk):
            """Score matmuls + the PSUM->SBUF rk multiply and pb add."""
            pbs = slice(pblk * 128, (pblk + 1) * 128)
            s_ps = psum3.tile([128, P], F32, tag="s_ps", name="s_ps", bufs=2)
            pb_sb = pb_tiles[pblk]
            for sl, s in halves:
                for dc in range(KC):
                    nc.tensor.matmul(
                        s_ps[:, s],
                        q1[:, dc, pbs],
                        k1[:, dc, s],
                        start=(dc == 0),
                        stop=False,
                    )
                for i in range(KC // 2):
                    c2 = slice(2 * i, 2 * i + 2)
                    nc.tensor.matmul(
                        s_ps[:, s],
                        dq8[:, c2, pbs],
                        k8[:, c2, s],
                        start=False, stop=False, perf_mode=PM.DoubleRow,
                    )
                for i in range(KC // 2):
                    c2 = slice(2 * i, 2 * i + 2)
                    nc.tensor.matmul(
                        s_ps[:, s],
                        q8[:, c2, pbs],
                        dk8[:, c2, s],
                        start=False, stop=(i == KC // 2 - 1),
                        perf_mode=PM.DoubleRow,
                    )
            if pblk == 0:
                emit_q_relayout()
            # pb * |q| runs off the critical chain (pb lands early via DMA)
            pb_q = work3.tile([128, P], F32, tag="pb_q")
            nc.scalar.activation(pb_q, pb_sb, AF.Identity,
                                 scale=nq_cols[:, pblk:pblk + 1])
            s_sb = work3.tile([128, P], F32, tag="s_sb")
            nc.vector.tensor_mul(s_sb, s_ps, rk_bcast)
            nc.gpsimd.tensor_add(s_sb, s_sb, pb_q)
            return s_sb

        def emit_scores_b(pblk, s_sb):
            """Top-16 + masked softmax + DMA transpose; returns (wT, rden).

            Emitted one block behind stage A so the DVE FIFO never has a
            stage-B op (waiting on Pool/ACT) ahead of the next block's
            PSUM-draining multiply.
            """
            # top-16 per row: two rounds of max8 + match_replace
            m8a = work3.tile([128, 8], F32, tag="m8a")
            nc.vector.max(m8a, s_sb)
            ebias = work3.tile([128, 1], F32, tag="ebias")
            nc.vector.tensor_mul(ebias, m8a[:, 0:1], rqTn[:, pblk:pblk + 1])
            sz1 = work3.tile([128, P], F32, tag="sz1")
            nc.vector.match_replace(sz1, in_to_replace=m8a, in_values=s_sb,
                                    imm_value=MINVAL)
            m8b = work3.tile([128, 8], F32, tag="m8b")
            nc.vector.max(m8b, sz1)
            sz2 = work3.tile([128, P], F32, tag="sz2")
            nc.vector.match_replace(sz2, in_to_replace=m8b, in_values=sz1,
                                    imm_value=MINVAL)
            # exp cancellation: w = exp((s''-m'')*rq/T) - exp((sz2-m'')*rq/T).
            # Off-top entries produce identical fp16 values and cancel
            # exactly; on-top entries of the second exp underflow to ~0.
            w_all = work3.tile([128, P], F16, tag="w_all")
            d_all = work3.tile([128, 1], F32, tag="d_all")
            nc.scalar.activation(w_all, s_sb, AF.Exp, bias=ebias,
                                 scale=rqT[:, pblk:pblk + 1], accum_out=d_all)
            w_z = work3.tile([128, P], F16, tag="w_z")
            d_z = work3.tile([128, 1], F32, tag="d_z")
            nc.scalar.activation(w_z, sz2, AF.Exp, bias=ebias,
                                 scale=rqT[:, pblk:pblk + 1], accum_out=d_z)
            w_sb = work3.tile([128, P], F16, tag="w_sb")
            nc.gpsimd.tensor_sub(w_sb, w_all, w_z)
            den = work3.tile([128, 1], F32, tag="den")
            nc.gpsimd.tensor_sub(den, d_all, d_z)
            rden = work3.tile([128, 1], F32, tag="rden", bufs=6)
            nc.vector.reciprocal(rden, den)
            # transpose w on the DMA xbar (off the PE)
            wT_sb = work3.tile([128, PB, 128], F16, tag="wT_sb", bufs=6)
            nc.scalar.dma_start_transpose(wT_sb, w_sb)
            return wT_sb, rden

        def emit_tail(pblk, wT_sb, rden):
            """PE tail: combine with v, scale, store."""
            pbs = slice(pblk * 128, (pblk + 1) * 128)
            o_ps = psum3.tile([128, D], F32, tag="o_ps", name="o_ps", bufs=1)
            for qc in range(PB):
                for sl, s in dhalves:
                    nc.tensor.matmul(
                        o_ps[:, s],
                        wT_sb[:, qc, :],
                        v_sb[:, qc, s],
                        start=(qc == 0),
                        stop=(qc == PB - 1),
                    )
            out_sb = work3.tile([128, D], F32, tag="out_sb")
            nc.scalar.activation(out_sb, o_ps, AF.Identity, scale=rden)
            nc.sync.dma_start(out[pbs, :], out_sb)

        # software pipeline: stage B (top-k chain) runs one block behind
        # stage A (matmuls), and block p's combine is emitted five A-stages
        # later, so the whole vector chain has ample slack before the PE
        # needs its weights.
        TLAG = 5
        sA = [None] * PB
        done = [None] * PB
        for pblk in range(PB):
            sA[pblk] = emit_scores_a(pblk)
            if pblk >= 1:
                done[pblk - 1] = emit_scores_b(pblk - 1, sA[pblk - 1])
            if pblk >= TLAG:
                emit_tail(pblk - TLAG, *done[pblk - TLAG])
        done[PB - 1] = emit_scores_b(PB - 1, sA[PB - 1])
        for pblk in range(PB - TLAG, PB):
            emit_tail(pblk, *done[pblk])

        work3.release()
        psum3.release()
        persist.release()
        consts.release()

    nc.finalize()
    return nc


_PROG_CACHE = {}


def _e5(x):
    return np.ascontiguousarray(np.asarray(x, np.float32).astype(
        ml_dtypes.float8_e5m2))


def kernel(**inputs) -> np.ndarray:
    x = np.ascontiguousarray(np.asarray(inputs["x"], dtype=np.float32))
    Wq = np.asarray(inputs["Wq"], dtype=np.float32)
    Wk = np.asarray(inputs["Wk"], dtype=np.float32)
    Wv = np.asarray(inputs["Wv"], dtype=np.float32)
    bq = np.asarray(inputs["bq"], dtype=np.float32)
    bk = np.asarray(inputs["bk"], dtype=np.float32)
    bv = np.asarray(inputs["bv"], dtype=np.float32)
    pos_bias = np.asarray(inputs["pos_bias"], dtype=np.float32)

    with_bias = bool(np.any(bq) or np.any(bk) or np.any(bv))

    # Diagonal is excluded by the reference (set to -1e9 before top-k); any
    # value below every real score gives the identical top-16 and weights.
    pb_adj = np.ascontiguousarray(pos_bias.copy())
    np.fill_diagonal(pb_adj, DIAGVAL)

    if with_bias not in _PROG_CACHE:
        _PROG_CACHE[with_bias] = build_program(with_bias)
    nc = _PROG_CACHE[with_bias]

    def wpack(W):
        W1 = W.astype(np.float16)
        dW = W - W1.astype(np.float32)
        return (np.ascontiguousarray(W1), _e5(W * (1.0 / SX)), _e5(dW * SW))

    Wq16, w8q, dw8q = wpack(Wq)
    Wk16, w8k, dw8k = wpack(Wk)
    Wv16 = np.ascontiguousarray(Wv.astype(np.float16))

    in_maps = []
    for b in range(B):
        xTb = np.ascontiguousarray(x[b, 1:, :].T)
        x16 = xTb.astype(np.float16)
        dxb = xTb - x16.astype(np.float32)
        m = {
            "xT": np.ascontiguousarray(x16),
            "x8_d": _e5(xTb * (1.0 / SW)),
            "dx8_d": _e5(dxb * SX),
            "Wq_d": Wq16, "Wk_d": Wk16, "Wv_d": Wv16,
            "w8q_d": w8q, "dw8q_d": dw8q, "w8k_d": w8k, "dw8k_d": dw8k,
            "pb": pb_adj,
        }
        if with_bias:
            m["bqkv"] = np.ascontiguousarray(np.stack([bq, bk, bv])[None])
        in_maps.append(m)

    res = run_bass_kernel_spmd(nc, in_maps, core_ids=list(range(B)))
    return np.stack([res.results[b]["out"] for b in range(B)]).astype(np.float32)
